# revision 3
# baseline (speedup 1.0000x reference)
"""AlphaFold2 axial (row/column) MSA attention on 8 Trainium2 NeuronCores.

Problem: x (1, 32768, 256) = 128 MSA rows x 256 columns x dim 256.
  - width attention: softmax attention across the 128 rows, independent per
    column (256 independent length-128 sequences), 8 heads x 64.
  - height attention: "tied" attention across the 256 columns: logits are
    summed over all 128 rows, one (256x256) softmax per head shared by all
    rows.

Sharding (8 cores):
  - width: each core owns 32 columns (fully local).
  - height: each core owns 16 rows; per-core partial logits (8,256,256) are
    AllReduce'd (bf16, 1MB) across cores, softmax replicated, attn*V local.

Layout strategy (everything bf16 into the PE, fp32 accumulation):
  - activations feature-major ("xT": features on partitions, tokens on free),
    prepared host-side, so projections and q.k^T need no on-device transpose.
  - scores are computed transposed, S^T = (j, i), by swapping matmul
    operands; softmax denominators are computed with an all-ones stationary
    matmul (partition-dim sum + broadcast in one PE op), normalization via
    reciprocal + multiply; no max-subtraction (logits are ~N(0, 0.1)).
  - attn*V consumes S^T directly and yields o^T feature-major, which feeds
    the output projection; outputs are written feature-major and transposed
    back on the host.
"""

import sys

for _p in ("/opt/trn_rl_repo",):
    if _p not in sys.path:
        sys.path.append(_p)

import numpy as np
import ml_dtypes

import concourse.bass as bass
import concourse.mybir as mybir
import concourse.tile as tile
from concourse import bacc
from concourse.bass_utils import run_bass_kernel_spmd

BF16 = mybir.dt.bfloat16
F32 = mybir.dt.float32
NPBF16 = ml_dtypes.bfloat16
EXP = mybir.ActivationFunctionType.Exp

N_CORES = 8
H_ROWS = 128          # MSA rows
W_COLS = 256          # sequence length (columns)
D = 256               # model dim
NH = 8                # heads
DH = 64               # head dim
INNER = NH * DH       # 512
WPC = W_COLS // N_CORES   # 32 columns per core
RPC = H_ROWS // N_CORES   # 16 rows per core
T = 4096              # tokens per shard (WPC*H_ROWS == RPC*W_COLS)
SCALE = DH ** -0.5                   # 0.125
TIE_SCALE = SCALE * (H_ROWS ** -0.5)


def _ap(h):
    return h.ap()


def build_bass():
    nc = bacc.Bacc("TRN2", target_bir_lowering=False, debug=False,
                   num_devices=N_CORES)

    # ---- per-core I/O ----
    xw = _ap(nc.dram_tensor("xw", [D, T], BF16, kind="ExternalInput"))
    xr = _ap(nc.dram_tensor("xr", [D, T], BF16, kind="ExternalInput"))
    wq = _ap(nc.dram_tensor("wq", [D, INNER], BF16, kind="ExternalInput"))
    wk = _ap(nc.dram_tensor("wk", [D, INNER], BF16, kind="ExternalInput"))
    wv = _ap(nc.dram_tensor("wv", [D, INNER], BF16, kind="ExternalInput"))
    wo = _ap(nc.dram_tensor("wo", [INNER, D], BF16, kind="ExternalInput"))
    hq = _ap(nc.dram_tensor("hq", [D, INNER], BF16, kind="ExternalInput"))
    hk = _ap(nc.dram_tensor("hk", [D, INNER], BF16, kind="ExternalInput"))
    hv = _ap(nc.dram_tensor("hv", [D, INNER], BF16, kind="ExternalInput"))
    ho = _ap(nc.dram_tensor("ho", [INNER, D], BF16, kind="ExternalInput"))
    w_out_t = _ap(nc.dram_tensor("w_out_t", [D, T], F32, kind="ExternalOutput"))
    h_out_t = _ap(nc.dram_tensor("h_out_t", [D, T], F32, kind="ExternalOutput"))

    # collective bounce buffers (height partial logits, bf16)
    cc_in = _ap(nc.dram_tensor("cc_in", [128, NH * 512], BF16, kind="Internal"))
    cc_out = _ap(nc.dram_tensor("cc_out", [128, NH * 512], BF16,
                                kind="Internal", addr_space="Shared"))

    with tile.TileContext(nc) as tc:
        build_tile_kernel(tc, xw, xr, wq, wk, wv, wo, hq, hk, hv, ho,
                          w_out_t, h_out_t, cc_in, cc_out)

    nc.compile()
    return nc


def build_tile_kernel(tc, xw, xr, wq, wk, wv, wo, hq, hk, hv, ho,
                      w_out_t, h_out_t, cc_in, cc_out):
    from contextlib import ExitStack

    nc = tc.nc
    ctx = ExitStack()

    consts = ctx.enter_context(tc.tile_pool(name="consts", bufs=1))

    # ---- constants / weights into SBUF ----
    def load_w2(ap_in, name):  # (256, 512) -> 2 chunks (128, 512)
        ts = []
        for kc in range(2):
            t = consts.tile([128, INNER], BF16, name=f"{name}{kc}")
            nc.sync.dma_start(out=t, in_=ap_in[kc * 128:(kc + 1) * 128, :])
            ts.append(t)
        return ts

    def load_w4(ap_in, name):  # (512, 256) -> 4 chunks (128, 256)
        ts = []
        for f in range(4):
            t = consts.tile([128, D], BF16, name=f"{name}{f}")
            nc.sync.dma_start(out=t, in_=ap_in[f * 128:(f + 1) * 128, :])
            ts.append(t)
        return ts

    wq_sb = load_w2(wq, "wq")
    wk_sb = load_w2(wk, "wk")
    wv_sb = load_w2(wv, "wv")
    hq_sb = load_w2(hq, "hq")
    hk_sb = load_w2(hk, "hk")
    hv_sb = load_w2(hv, "hv")
    wo_sb = load_w4(wo, "wo")
    ho_sb = load_w4(ho, "ho")

    ones_sb = consts.tile([128, 128], BF16, name="ones")
    nc.vector.memset(ones_sb, 1.0)

    xr_sb = []
    for kc in range(2):
        t = consts.tile([128, T], BF16, name=f"xr{kc}")
        nc.sync.dma_start(out=t, in_=xr[kc * 128:(kc + 1) * 128, :])
        xr_sb.append(t)
    xw_sb = []
    for kc in range(2):
        t = consts.tile([128, T], BF16, name=f"xw{kc}")
        nc.sync.dma_start(out=t, in_=xw[kc * 128:(kc + 1) * 128, :])
        xw_sb.append(t)

    # ---------------------------------------------------------------
    # Phase A: height q/k projections + partial tied logits; AllReduce.
    # dots^T[H](j, i) = sum_r sum_d k[r,j,H,d] q[r,i,H,d]  (j,i = columns)
    # ---------------------------------------------------------------
    with tc.tile_pool(name="phaseA", bufs=1) as phaseA, \
         tc.tile_pool(name="psA", bufs=3, space="PSUM") as psA, \
         tc.tile_pool(name="psDA", bufs=2, space="PSUM") as psDA:

        def project_fmajor(w_sb, x_sb, pool, name):
            """(feat, tok) = w^T @ x^T -> 4 chunks (128, T) bf16."""
            outs = []
            for f in range(4):
                t = pool.tile([128, T], BF16, name=f"{name}{f}")
                outs.append(t)
                for nt in range(T // 512):
                    ps = psA.tile([128, 512], F32, tag="projA", name="projA")
                    for kc in range(2):
                        nc.tensor.matmul(
                            out=ps,
                            lhsT=w_sb[kc][:, f * 128:(f + 1) * 128],
                            rhs=x_sb[kc][:, nt * 512:(nt + 1) * 512],
                            start=(kc == 0), stop=(kc == 1))
                    nc.any.tensor_copy(out=t[:, nt * 512:(nt + 1) * 512],
                                       in_=ps)
            return outs

        qhT = project_fmajor(hq_sb, xr_sb, phaseA, "qhT")
        khT = project_fmajor(hk_sb, xr_sb, phaseA, "khT")

        # partial dots^T, bf16: (128, [H][jc][i]) free = H*512 + jc*256 + i
        dots_sb = phaseA.tile([128, NH * 512], BF16, name="dots_sb")
        for f in range(4):
            # heads 2f (free 0:512, bank 0) and 2f+1 (free 512:1024, bank 1)
            dps = psDA.tile([128, 1024], F32, tag="hdots", name="hdots")
            # NB: complete each (jc) accumulation chain before starting the
            # next one in the same PSUM bank — matmul start=True clears
            # has_written for the whole bank, so interleaved chains corrupt.
            for jc in range(2):
                for r in range(RPC):
                    for hp in range(2):
                        b = hp * 64
                        nc.tensor.matmul(
                            out=dps[:, hp * 512 + jc * 256:
                                    hp * 512 + (jc + 1) * 256],
                            lhsT=khT[f][b:b + 64, r * 256 + jc * 128:
                                        r * 256 + jc * 128 + 128],
                            rhs=qhT[f][b:b + 64, r * 256:(r + 1) * 256],
                            start=(r == 0), stop=(r == RPC - 1))
            for hp in range(2):
                H = 2 * f + hp
                nc.any.tensor_copy(out=dots_sb[:, H * 512:(H + 1) * 512],
                                   in_=dps[:, hp * 512:(hp + 1) * 512])

        nc.sync.dma_start(out=cc_in[:, :], in_=dots_sb[:, :])
        nc.gpsimd.collective_compute(
            "AllReduce", mybir.AluOpType.add,
            replica_groups=[list(range(N_CORES))],
            ins=[cc_in.opt()], outs=[cc_out.opt()])

    # ---------------------------------------------------------------
    # Phase B: width attention over this core's 32 columns.
    # Ew slot layout: slot(H) = (H%2)*512 + (H//2)*128
    # ---------------------------------------------------------------
    NCG = 8                      # columns per group
    NGRP = WPC // NCG            # 4 groups
    GT = NCG * 128               # tokens per group (1024)

    with tc.tile_pool(name="phaseB", bufs=1) as phaseB, \
         tc.tile_pool(name="grpB", bufs=2) as grpB, \
         tc.tile_pool(name="colB", bufs=3) as colB, \
         tc.tile_pool(name="stgB", bufs=3) as stgB, \
         tc.tile_pool(name="psB", bufs=2, space="PSUM") as psB, \
         tc.tile_pool(name="psW", bufs=1, space="PSUM") as psW, \
         tc.tile_pool(name="psS", bufs=2, space="PSUM") as psS, \
         tc.tile_pool(name="psO", bufs=2, space="PSUM") as psO:

        owT = [phaseB.tile([128, T], BF16, name=f"owT{f}") for f in range(4)]

        for g in range(NGRP):
            tok0 = g * GT
            # group-local q^T, k^T (feature-major) and v (token-major)
            qwT, kwT = [], []
            for f in range(4):
                for which, (w_sb, lst) in enumerate(
                        ((wq_sb, qwT), (wk_sb, kwT))):
                    t = grpB.tile([128, GT], BF16, tag=f"qk{which}{f}",
                                  name=f"qk{which}{f}")
                    lst.append(t)
                    for nt in range(GT // 512):
                        ps = psB.tile([128, 512], F32, tag="projB",
                                      name="projB")
                        for kc in range(2):
                            nc.tensor.matmul(
                                out=ps,
                                lhsT=w_sb[kc][:, f * 128:(f + 1) * 128],
                                rhs=xw_sb[kc][:, tok0 + nt * 512:
                                              tok0 + (nt + 1) * 512],
                                start=(kc == 0), stop=(kc == 1))
                        nc.any.tensor_copy(
                            out=t[:, nt * 512:(nt + 1) * 512], in_=ps)
            vw = []
            for ci in range(NCG):
                t = grpB.tile([128, INNER], BF16, tag=f"vw{ci}",
                              name=f"vw{ci}")
                vw.append(t)
                ps = psB.tile([128, 512], F32, tag="projB", name="projB")
                for kc in range(2):
                    nc.tensor.matmul(
                        out=ps,
                        lhsT=xw_sb[kc][:, tok0 + ci * 128:
                                       tok0 + (ci + 1) * 128],
                        rhs=wv_sb[kc],
                        start=(kc == 0), stop=(kc == 1))
                nc.any.tensor_copy(out=t, in_=ps)

            for ci in range(NCG):
                c0 = ci * 128  # token offset within group
                # scores^T: S^T[H=2f+hp] at free hp*512 + f*128
                dpsW = psW.tile([128, 1024], F32, tag="wdots", name="wdots")
                for f in range(4):
                    for hp in range(2):
                        b = hp * 64
                        nc.tensor.matmul(
                            out=dpsW[:, hp * 512 + f * 128:
                                     hp * 512 + (f + 1) * 128],
                            lhsT=kwT[f][b:b + 64, c0:c0 + 128],
                            rhs=qwT[f][b:b + 64, c0:c0 + 128],
                            start=True, stop=True)
                Ew = colB.tile([128, 1024], BF16, tag="Ew", name="Ew")
                for hp in range(2):
                    nc.scalar.activation(
                        out=Ew[:, hp * 512:(hp + 1) * 512],
                        in_=dpsW[:, hp * 512:(hp + 1) * 512],
                        func=EXP, scale=SCALE)
                # denominators broadcast to all partitions (ones-matmul)
                Binv = colB.tile([128, 1024], F32, tag="Binv", name="Binv")
                for hp in range(2):
                    bps = psS.tile([128, 512], F32, tag="bsum", name="bsum")
                    nc.tensor.matmul(out=bps, lhsT=ones_sb,
                                     rhs=Ew[:, hp * 512:(hp + 1) * 512],
                                     start=True, stop=True)
                    nc.vector.reciprocal_approx_fast(
                        out=Binv[:, hp * 512:(hp + 1) * 512], in_=bps)
                EwN = colB.tile([128, 1024], BF16, tag="EwN", name="EwN")
                nc.gpsimd.tensor_mul(out=EwN, in0=Ew, in1=Binv)
                # attn * V -> o^T chunks (f: head 2f @ part 0:64, 2f+1 @ 64:128)
                ops = psO.tile([128, 512], F32, tag="opsW", name="opsW")
                for f in range(4):
                    for hp in range(2):
                        H = 2 * f + hp
                        slot = (H % 2) * 512 + (H // 2) * 128
                        nc.tensor.matmul(
                            out=ops[hp * 64:hp * 64 + 64,
                                    f * 128:(f + 1) * 128],
                            lhsT=vw[ci][:, H * 64:(H + 1) * 64],
                            rhs=EwN[:, slot:slot + 128],
                            start=True, stop=True)
                for f in range(4):
                    nc.any.tensor_copy(
                        out=owT[f][:, tok0 + c0:tok0 + c0 + 128],
                        in_=ops[:, f * 128:(f + 1) * 128])

        # width output projection: w_out^T = wo^T @ o^T
        for mc in range(2):
            for nt in range(T // 512):
                ps = psB.tile([128, 512], F32, tag="projB", name="projB")
                for f in range(4):
                    nc.tensor.matmul(
                        out=ps,
                        lhsT=wo_sb[f][:, mc * 128:(mc + 1) * 128],
                        rhs=owT[f][:, nt * 512:(nt + 1) * 512],
                        start=(f == 0), stop=(f == 3))
                st = stgB.tile([128, 512], F32, tag="stgW", name="stgW")
                nc.any.tensor_copy(out=st, in_=ps)
                nc.sync.dma_start(
                    out=w_out_t[mc * 128:(mc + 1) * 128,
                                nt * 512:(nt + 1) * 512],
                    in_=st)

    # ---------------------------------------------------------------
    # Phase C: height attention finish (after AllReduce).
    # ---------------------------------------------------------------
    with tc.tile_pool(name="phaseC", bufs=1) as phaseC, \
         tc.tile_pool(name="stgC", bufs=3) as stgC, \
         tc.tile_pool(name="psC", bufs=2, space="PSUM") as psC, \
         tc.tile_pool(name="psSC", bufs=2, space="PSUM") as psSC, \
         tc.tile_pool(name="psOC", bufs=2, space="PSUM") as psOC:

        # v (token-major) for the row shard: 32 chunks (128, 512)
        vh = []
        for rc in range(32):
            t = phaseC.tile([128, INNER], BF16, name=f"vh{rc}")
            vh.append(t)
            ps = psC.tile([128, 512], F32, tag="projC", name="projC")
            for kc in range(2):
                nc.tensor.matmul(
                    out=ps,
                    lhsT=xr_sb[kc][:, rc * 128:(rc + 1) * 128],
                    rhs=hv_sb[kc],
                    start=(kc == 0), stop=(kc == 1))
            nc.any.tensor_copy(out=t, in_=ps)

        dotsr = phaseC.tile([128, NH * 512], BF16, name="dotsr")
        nc.sync.dma_start(out=dotsr[:, :], in_=cc_out[:, :])

        Eh = phaseC.tile([128, NH * 512], BF16, name="Eh")
        for H in range(NH):
            nc.scalar.activation(out=Eh[:, H * 512:(H + 1) * 512],
                                 in_=dotsr[:, H * 512:(H + 1) * 512],
                                 func=EXP, scale=TIE_SCALE)
        # denominators: B_H(i) = sum over both j-chunks and partitions
        BinvH = phaseC.tile([128, NH * 256], F32, name="BinvH")
        for f in range(4):
            bps = psSC.tile([128, 512], F32, tag="bsumH", name="bsumH")
            for hp in range(2):
                H = 2 * f + hp
                for jc in range(2):
                    nc.tensor.matmul(
                        out=bps[:, hp * 256:(hp + 1) * 256],
                        lhsT=ones_sb,
                        rhs=Eh[:, H * 512 + jc * 256:
                               H * 512 + (jc + 1) * 256],
                        start=(jc == 0), stop=(jc == 1))
            nc.vector.reciprocal_approx_fast(
                out=BinvH[:, f * 512:(f + 1) * 512], in_=bps)
        EhN = phaseC.tile([128, NH * 512], BF16, name="EhN")
        for H in range(NH):
            for jc in range(2):
                nc.gpsimd.tensor_mul(
                    out=EhN[:, H * 512 + jc * 256: H * 512 + (jc + 1) * 256],
                    in0=Eh[:, H * 512 + jc * 256: H * 512 + (jc + 1) * 256],
                    in1=BinvH[:, H * 256:(H + 1) * 256])

        # attn * V per row -> o^T chunks; ohT[f] free = r*256 + i
        ohT = [phaseC.tile([128, T], BF16, name=f"ohT{f}") for f in range(4)]
        for r in range(RPC):
            ops = psOC.tile([128, 1024], F32, tag="opsH", name="opsH")
            for f in range(4):
                for hp in range(2):
                    H = 2 * f + hp
                    for jc in range(2):
                        nc.tensor.matmul(
                            out=ops[hp * 64:hp * 64 + 64,
                                    f * 256:(f + 1) * 256],
                            lhsT=vh[r * 2 + jc][:, H * 64:(H + 1) * 64],
                            rhs=EhN[:, H * 512 + jc * 256:
                                    H * 512 + (jc + 1) * 256],
                            start=(jc == 0), stop=(jc == 1))
            for f in range(4):
                nc.any.tensor_copy(out=ohT[f][:, r * 256:(r + 1) * 256],
                                   in_=ops[:, f * 256:(f + 1) * 256])

        # height output projection
        for mc in range(2):
            for nt in range(T // 512):
                ps = psC.tile([128, 512], F32, tag="projC", name="projC")
                for f in range(4):
                    nc.tensor.matmul(
                        out=ps,
                        lhsT=ho_sb[f][:, mc * 128:(mc + 1) * 128],
                        rhs=ohT[f][:, nt * 512:(nt + 1) * 512],
                        start=(f == 0), stop=(f == 3))
                st = stgC.tile([128, 512], F32, tag="stgH", name="stgH")
                nc.any.tensor_copy(out=st, in_=ps)
                nc.sync.dma_start(
                    out=h_out_t[mc * 128:(mc + 1) * 128,
                                nt * 512:(nt + 1) * 512],
                    in_=st)

    ctx.close()


_NC = None


def _get_nc():
    global _NC
    if _NC is None:
        _NC = build_bass()
    return _NC


def make_in_maps(x, wq_w, wkv_w, wout_w, hq_w, hkv_w, hout_w):
    x4 = np.asarray(x, np.float32).reshape(H_ROWS, W_COLS, D)
    xb = x4.astype(NPBF16)
    wghts = {
        "wq": np.ascontiguousarray(np.asarray(wq_w, np.float32).astype(NPBF16)),
        "wk": np.ascontiguousarray(np.asarray(wkv_w, np.float32)[:, :INNER].astype(NPBF16)),
        "wv": np.ascontiguousarray(np.asarray(wkv_w, np.float32)[:, INNER:].astype(NPBF16)),
        "wo": np.ascontiguousarray(np.asarray(wout_w, np.float32).astype(NPBF16)),
        "hq": np.ascontiguousarray(np.asarray(hq_w, np.float32).astype(NPBF16)),
        "hk": np.ascontiguousarray(np.asarray(hkv_w, np.float32)[:, :INNER].astype(NPBF16)),
        "hv": np.ascontiguousarray(np.asarray(hkv_w, np.float32)[:, INNER:].astype(NPBF16)),
        "ho": np.ascontiguousarray(np.asarray(hout_w, np.float32).astype(NPBF16)),
    }
    in_maps = []
    for c in range(N_CORES):
        xw_c = np.ascontiguousarray(
            xb[:, c * WPC:(c + 1) * WPC, :].transpose(1, 0, 2)
            .reshape(T, D).T)
        xr_c = np.ascontiguousarray(xb[c * RPC:(c + 1) * RPC].reshape(T, D).T)
        m = {"xw": xw_c, "xr": xr_c}
        m.update(wghts)
        in_maps.append(m)
    return in_maps


def assemble_output(results, wout_b, hout_b):
    w_full = np.empty((H_ROWS, W_COLS, D), np.float32)
    h_full = np.empty((H_ROWS, W_COLS, D), np.float32)
    for c in range(N_CORES):
        wt = results[c]["w_out_t"]  # (256, 4096)
        w_full[:, c * WPC:(c + 1) * WPC, :] = \
            wt.T.reshape(WPC, H_ROWS, D).transpose(1, 0, 2)
        ht = results[c]["h_out_t"]
        h_full[c * RPC:(c + 1) * RPC] = ht.T.reshape(RPC, W_COLS, D)
    out = w_full + h_full
    out += (np.asarray(wout_b, np.float32) + np.asarray(hout_b, np.float32))
    return out.reshape(1, H_ROWS * W_COLS, D)


def kernel(x, wq_w, wkv_w, wout_w, wout_b, hq_w, hkv_w, hout_w, hout_b,
           msa_h=H_ROWS, msa_w=W_COLS, **_unused):
    in_maps = make_in_maps(x, wq_w, wkv_w, wout_w, hq_w, hkv_w, hout_w)
    nc = _get_nc()
    res = run_bass_kernel_spmd(nc, in_maps, core_ids=list(range(N_CORES)))
    return assemble_output(res.results, wout_b, hout_b)


# revision 5
# speedup vs baseline: 268.4386x; 268.4386x over previous
"""AlphaFold2 axial (row/column) MSA attention on 8 Trainium2 NeuronCores.

Problem: x (1, 32768, 256) = 128 MSA rows x 256 columns x dim 256.
  - width attention: softmax attention across the 128 rows, independent per
    column (256 independent length-128 sequences), 8 heads x 64.
  - height attention: "tied" attention across the 256 columns: logits are
    summed over all 128 rows, one (256x256) softmax per head shared by all
    rows.

Sharding (8 cores):
  - width: each core owns 32 columns (fully local).
  - height: each core owns 16 rows; per-core partial logits (8,256,256) are
    AllReduce'd (bf16, 1MB) across cores, softmax replicated, attn*V local.

Layout strategy (everything bf16 into the PE, fp32 accumulation):
  - activations feature-major ("xT": features on partitions, tokens on free),
    prepared host-side, so projections and q.k^T need no on-device transpose.
  - scores are computed transposed, S^T = (j, i), by swapping matmul
    operands; softmax denominators are computed with an all-ones stationary
    matmul (partition-dim sum + broadcast in one PE op), normalization via
    reciprocal + multiply; no max-subtraction (logits are ~N(0, 0.1)).
  - attn*V consumes S^T directly and yields o^T feature-major, which feeds
    the output projection; outputs are written feature-major and transposed
    back on the host.

PSUM rules honored here: a matmul accumulation chain must fully finish
before another chain's start=True touches the same PSUM bank (start clears
has_written bank-wide; data values persist).
"""

import sys

for _p in ("/opt/trn_rl_repo",):
    if _p not in sys.path:
        sys.path.append(_p)

import numpy as np
import ml_dtypes

import concourse.bass as bass
import concourse.mybir as mybir
import concourse.tile as tile
from concourse import bacc
from concourse.bass_utils import run_bass_kernel_spmd

BF16 = mybir.dt.bfloat16
F32 = mybir.dt.float32
NPBF16 = ml_dtypes.bfloat16
EXP = mybir.ActivationFunctionType.Exp

N_CORES = 8
H_ROWS = 128          # MSA rows
W_COLS = 256          # sequence length (columns)
D = 256               # model dim
NH = 8                # heads
DH = 64               # head dim
INNER = NH * DH       # 512
WPC = W_COLS // N_CORES   # 32 columns per core
RPC = H_ROWS // N_CORES   # 16 rows per core
T = 4096              # tokens per shard (WPC*H_ROWS == RPC*W_COLS)
SCALE = DH ** -0.5                   # 0.125
TIE_SCALE = SCALE * (H_ROWS ** -0.5)


def _ap(h):
    return h.ap()


def build_bass(loop=1):
    nc = bacc.Bacc("TRN2", target_bir_lowering=False, debug=False,
                   num_devices=N_CORES)

    # ---- per-core I/O ----
    xw = _ap(nc.dram_tensor("xw", [D, T], BF16, kind="ExternalInput"))
    xr = _ap(nc.dram_tensor("xr", [D, T], BF16, kind="ExternalInput"))
    wq = _ap(nc.dram_tensor("wq", [D, INNER], BF16, kind="ExternalInput"))
    wk = _ap(nc.dram_tensor("wk", [D, INNER], BF16, kind="ExternalInput"))
    wv = _ap(nc.dram_tensor("wv", [D, INNER], BF16, kind="ExternalInput"))
    wo = _ap(nc.dram_tensor("wo", [INNER, D], BF16, kind="ExternalInput"))
    hq = _ap(nc.dram_tensor("hq", [D, INNER], BF16, kind="ExternalInput"))
    hk = _ap(nc.dram_tensor("hk", [D, INNER], BF16, kind="ExternalInput"))
    hv = _ap(nc.dram_tensor("hv", [D, INNER], BF16, kind="ExternalInput"))
    ho = _ap(nc.dram_tensor("ho", [INNER, D], BF16, kind="ExternalInput"))
    w_out_t = _ap(nc.dram_tensor("w_out_t", [D, T], F32, kind="ExternalOutput"))
    h_out_t = _ap(nc.dram_tensor("h_out_t", [D, T], F32, kind="ExternalOutput"))

    with tile.TileContext(nc) as tc:
        for it in range(loop):
            # collective buffers must be distinct per unrolled iteration
            cc_in = _ap(nc.dram_tensor(f"cc_in{it}", [128, NH * 512], BF16,
                                       kind="Internal"))
            cc_out = _ap(nc.dram_tensor(f"cc_out{it}", [128, NH * 512], BF16,
                                        kind="Internal", addr_space="Shared"))
            build_tile_kernel(tc, xw, xr, wq, wk, wv, wo, hq, hk, hv, ho,
                              w_out_t, h_out_t, cc_in, cc_out)

    nc.compile()
    return nc


def build_tile_kernel(tc, xw, xr, wq, wk, wv, wo, hq, hk, hv, ho,
                      w_out_t, h_out_t, cc_in, cc_out):
    from contextlib import ExitStack

    nc = tc.nc
    ctx = ExitStack()

    # round-robin PSUM->SBUF copy engine (only ACT/DVE can read PSUM)
    _eng = [0]

    def copy_ps(out, in_):
        _eng[0] ^= 1
        (nc.scalar.copy if _eng[0] else nc.vector.tensor_copy)(out=out, in_=in_)

    consts = ctx.enter_context(tc.tile_pool(name="consts", bufs=1))

    # ---- constants / weights into SBUF ----
    def load_w2(ap_in, name):  # (256, 512) -> 2 chunks (128, 512)
        ts = []
        for kc in range(2):
            t = consts.tile([128, INNER], BF16, name=f"{name}{kc}")
            nc.sync.dma_start(out=t, in_=ap_in[kc * 128:(kc + 1) * 128, :])
            ts.append(t)
        return ts

    def load_w4(ap_in, name):  # (512, 256) -> 4 chunks (128, 256)
        ts = []
        for f in range(4):
            t = consts.tile([128, D], BF16, name=f"{name}{f}")
            nc.sync.dma_start(out=t, in_=ap_in[f * 128:(f + 1) * 128, :])
            ts.append(t)
        return ts

    # phase-A dependencies first so the PE can start ASAP
    hq_sb = load_w2(hq, "hq")
    hk_sb = load_w2(hk, "hk")
    xr_sb = []
    for kc in range(2):
        t = consts.tile([128, T], BF16, name=f"xr{kc}")
        nc.sync.dma_start(out=t, in_=xr[kc * 128:(kc + 1) * 128, :])
        xr_sb.append(t)
    xw_sb = []
    for kc in range(2):
        t = consts.tile([128, T], BF16, name=f"xw{kc}")
        nc.sync.dma_start(out=t, in_=xw[kc * 128:(kc + 1) * 128, :])
        xw_sb.append(t)
    wq_sb = load_w2(wq, "wq")
    wk_sb = load_w2(wk, "wk")
    wv_sb = load_w2(wv, "wv")
    hv_sb = load_w2(hv, "hv")
    wo_sb = load_w4(wo, "wo")
    ho_sb = load_w4(ho, "ho")

    ones_sb = consts.tile([128, 128], BF16, name="ones")
    nc.vector.memset(ones_sb, 1.0)

    # ---------------------------------------------------------------
    # Phase A: height q/k projections + partial tied logits; AllReduce.
    # dots^T[H](j, i) = sum_r sum_d k[r,j,H,d] q[r,i,H,d]  (j,i = columns)
    # ---------------------------------------------------------------
    with tc.tile_pool(name="phaseA", bufs=1) as phaseA, \
         tc.tile_pool(name="psA", bufs=3, space="PSUM") as psA, \
         tc.tile_pool(name="psDA", bufs=2, space="PSUM") as psDA:

        def project_fmajor(w_sb, x_sb, pool, name):
            """(feat, tok) = w^T @ x^T -> 4 chunks (128, T) bf16."""
            outs = []
            for f in range(4):
                t = pool.tile([128, T], BF16, name=f"{name}{f}")
                outs.append(t)
                for nt in range(T // 512):
                    ps = psA.tile([128, 512], F32, tag="projA", name="projA")
                    for kc in range(2):
                        nc.tensor.matmul(
                            out=ps,
                            lhsT=w_sb[kc][:, f * 128:(f + 1) * 128],
                            rhs=x_sb[kc][:, nt * 512:(nt + 1) * 512],
                            start=(kc == 0), stop=(kc == 1))
                    copy_ps(t[:, nt * 512:(nt + 1) * 512], ps)
            return outs

        qhT = project_fmajor(hq_sb, xr_sb, phaseA, "qhT")
        khT = project_fmajor(hk_sb, xr_sb, phaseA, "khT")

        # partial dots^T, bf16: (128, [H][jc][i]) free = H*512 + jc*256 + i
        dots_sb = phaseA.tile([128, NH * 512], BF16, name="dots_sb")
        for f in range(4):
            # heads 2f (free 0:512, bank 0) and 2f+1 (free 512:1024, bank 1)
            dps = psDA.tile([128, 1024], F32, tag="hdots", name="hdots")
            # complete each jc accumulation chain before the next starts in
            # the same bank (start=True clears has_written bank-wide)
            for jc in range(2):
                for r in range(RPC):
                    for hp in range(2):
                        b = hp * 64
                        nc.tensor.matmul(
                            out=dps[:, hp * 512 + jc * 256:
                                    hp * 512 + (jc + 1) * 256],
                            lhsT=khT[f][b:b + 64, r * 256 + jc * 128:
                                        r * 256 + jc * 128 + 128],
                            rhs=qhT[f][b:b + 64, r * 256:(r + 1) * 256],
                            start=(r == 0), stop=(r == RPC - 1))
            copy_ps(dots_sb[:, 2 * f * 512:(2 * f + 2) * 512], dps)

        nc.sync.dma_start(out=cc_in[:, :], in_=dots_sb[:, :])
        nc.gpsimd.collective_compute(
            "AllReduce", mybir.AluOpType.add,
            replica_groups=[list(range(N_CORES))],
            ins=[cc_in.opt()], outs=[cc_out.opt()])

    # ---------------------------------------------------------------
    # Phase B: width attention over this core's 32 columns.
    # Ew slot layout: slot(H) = (H%2)*512 + (H//2)*128
    # ---------------------------------------------------------------
    NCG = 8                      # columns per group
    NGRP = WPC // NCG            # 4 groups
    GT = NCG * 128               # tokens per group (1024)

    with tc.tile_pool(name="phaseB", bufs=1) as phaseB, \
         tc.tile_pool(name="grpB", bufs=2) as grpB, \
         tc.tile_pool(name="colB", bufs=4) as colB, \
         tc.tile_pool(name="stgB", bufs=3) as stgB, \
         tc.tile_pool(name="psB", bufs=2, space="PSUM") as psB, \
         tc.tile_pool(name="psW", bufs=2, space="PSUM") as psW, \
         tc.tile_pool(name="psSO", bufs=2, space="PSUM") as psSO:

        # o^T accumulator: (128, f, tok) -- chunk f holds heads 2f, 2f+1
        owT = phaseB.tile([128, 4, T], BF16, name="owT")

        for g in range(NGRP):
            tok0 = g * GT
            # group-local q^T, k^T (feature-major) and v (token-major)
            qwT, kwT = [], []
            for f in range(4):
                for which, (w_sb, lst) in enumerate(
                        ((wq_sb, qwT), (wk_sb, kwT))):
                    t = grpB.tile([128, GT], BF16, tag=f"qk{which}{f}",
                                  name=f"qk{which}{f}")
                    lst.append(t)
                    for nt in range(GT // 512):
                        ps = psB.tile([128, 512], F32, tag="projB",
                                      name="projB")
                        for kc in range(2):
                            nc.tensor.matmul(
                                out=ps,
                                lhsT=w_sb[kc][:, f * 128:(f + 1) * 128],
                                rhs=xw_sb[kc][:, tok0 + nt * 512:
                                              tok0 + (nt + 1) * 512],
                                start=(kc == 0), stop=(kc == 1))
                        copy_ps(t[:, nt * 512:(nt + 1) * 512], ps)
            vw = []
            for ci in range(NCG):
                t = grpB.tile([128, INNER], BF16, tag=f"vw{ci}",
                              name=f"vw{ci}")
                vw.append(t)
                ps = psB.tile([128, 512], F32, tag="projB", name="projB")
                for kc in range(2):
                    nc.tensor.matmul(
                        out=ps,
                        lhsT=xw_sb[kc][:, tok0 + ci * 128:
                                       tok0 + (ci + 1) * 128],
                        rhs=wv_sb[kc],
                        start=(kc == 0), stop=(kc == 1))
                copy_ps(t, ps)

            for ci in range(NCG):
                c0 = ci * 128  # token offset within group
                # scores^T: S^T[H=2f+hp] at free hp*512 + f*128
                dpsW = psW.tile([128, 1024], F32, tag="wdots", name="wdots")
                for f in range(4):
                    for hp in range(2):
                        b = hp * 64
                        nc.tensor.matmul(
                            out=dpsW[:, hp * 512 + f * 128:
                                     hp * 512 + (f + 1) * 128],
                            lhsT=kwT[f][b:b + 64, c0:c0 + 128],
                            rhs=qwT[f][b:b + 64, c0:c0 + 128],
                            start=True, stop=True)
                Ew = colB.tile([128, 1024], BF16, tag="Ew", name="Ew")
                Binv = colB.tile([128, 1024], F32, tag="Binv", name="Binv")
                EwN = colB.tile([128, 1024], BF16, tag="EwN", name="EwN")
                ops = psSO.tile([128, 512], F32, tag="opsW", name="opsW")
                for hp in range(2):
                    hb = hp * 512
                    nc.scalar.activation(
                        out=Ew[:, hb:hb + 512], in_=dpsW[:, hb:hb + 512],
                        func=EXP, scale=SCALE)
                    # denominator sums broadcast to all partitions, written
                    # in-place over the consumed scores bank
                    nc.tensor.matmul(out=dpsW[:, hb:hb + 512], lhsT=ones_sb,
                                     rhs=Ew[:, hb:hb + 512],
                                     start=True, stop=True)
                    nc.vector.reciprocal_approx_fast(
                        out=Binv[:, hb:hb + 512], in_=dpsW[:, hb:hb + 512])
                    nc.gpsimd.tensor_mul(out=EwN[:, hb:hb + 512],
                                         in0=Ew[:, hb:hb + 512],
                                         in1=Binv[:, hb:hb + 512])
                    # attn * V for the 4 heads of this half
                    # (o^T chunk f: head 2f @ part 0:64, 2f+1 @ 64:128)
                    for f in range(4):
                        H = 2 * f + hp
                        nc.tensor.matmul(
                            out=ops[hp * 64:hp * 64 + 64,
                                    f * 128:(f + 1) * 128],
                            lhsT=vw[ci][:, H * 64:(H + 1) * 64],
                            rhs=EwN[:, hb + f * 128:hb + (f + 1) * 128],
                            start=True, stop=True)
                copy_ps(owT[:, :, tok0 + c0:tok0 + c0 + 128],
                        ops.rearrange("p (f i) -> p f i", f=4))

        # width output projection: w_out^T = wo^T @ o^T
        for mc in range(2):
            for nt in range(T // 512):
                ps = psW.tile([128, 512], F32, tag="wdots", name="oprojW")
                for f in range(4):
                    nc.tensor.matmul(
                        out=ps,
                        lhsT=wo_sb[f][:, mc * 128:(mc + 1) * 128],
                        rhs=owT[:, f, nt * 512:(nt + 1) * 512],
                        start=(f == 0), stop=(f == 3))
                st = stgB.tile([128, 512], F32, tag="stgW", name="stgW")
                copy_ps(st, ps)
                nc.sync.dma_start(
                    out=w_out_t[mc * 128:(mc + 1) * 128,
                                nt * 512:(nt + 1) * 512],
                    in_=st)

    # ---------------------------------------------------------------
    # Phase C: height attention finish (after AllReduce).
    # ---------------------------------------------------------------
    with tc.tile_pool(name="phaseC", bufs=1) as phaseC, \
         tc.tile_pool(name="stgC", bufs=3) as stgC, \
         tc.tile_pool(name="psC", bufs=2, space="PSUM") as psC, \
         tc.tile_pool(name="psSC", bufs=2, space="PSUM") as psSC, \
         tc.tile_pool(name="psOC", bufs=2, space="PSUM") as psOC:

        # v (token-major) for the row shard: 32 chunks (128, 512)
        vh = []
        for rc in range(32):
            t = phaseC.tile([128, INNER], BF16, name=f"vh{rc}")
            vh.append(t)
            ps = psC.tile([128, 512], F32, tag="projC", name="projC")
            for kc in range(2):
                nc.tensor.matmul(
                    out=ps,
                    lhsT=xr_sb[kc][:, rc * 128:(rc + 1) * 128],
                    rhs=hv_sb[kc],
                    start=(kc == 0), stop=(kc == 1))
            copy_ps(t, ps)

        dotsr = phaseC.tile([128, NH * 512], BF16, name="dotsr")
        nc.sync.dma_start(out=dotsr[:, :], in_=cc_out[:, :])

        Eh = phaseC.tile([128, NH * 512], BF16, name="Eh")
        for H in range(NH):
            nc.scalar.activation(out=Eh[:, H * 512:(H + 1) * 512],
                                 in_=dotsr[:, H * 512:(H + 1) * 512],
                                 func=EXP, scale=TIE_SCALE)
        # denominators: B_H(i) = sum over both j-chunks and partitions
        BinvH = phaseC.tile([128, NH * 256], F32, name="BinvH")
        for f in range(4):
            bps = psSC.tile([128, 512], F32, tag="bsumH", name="bsumH")
            for hp in range(2):
                H = 2 * f + hp
                for jc in range(2):
                    nc.tensor.matmul(
                        out=bps[:, hp * 256:(hp + 1) * 256],
                        lhsT=ones_sb,
                        rhs=Eh[:, H * 512 + jc * 256:
                               H * 512 + (jc + 1) * 256],
                        start=(jc == 0), stop=(jc == 1))
                nc.vector.reciprocal_approx_fast(
                    out=BinvH[:, H * 256:(H + 1) * 256],
                    in_=bps[:, hp * 256:(hp + 1) * 256])
        EhN = phaseC.tile([128, NH * 512], BF16, name="EhN")
        for H in range(NH):
            for jc in range(2):
                nc.gpsimd.tensor_mul(
                    out=EhN[:, H * 512 + jc * 256: H * 512 + (jc + 1) * 256],
                    in0=Eh[:, H * 512 + jc * 256: H * 512 + (jc + 1) * 256],
                    in1=BinvH[:, H * 256:(H + 1) * 256])

        # attn * V per row -> o^T chunks; ohT free = (f, r*256 + i)
        ohT = phaseC.tile([128, 4, T], BF16, name="ohT")
        for r in range(RPC):
            ops = psOC.tile([128, 1024], F32, tag="opsH", name="opsH")
            for f in range(4):
                for hp in range(2):
                    H = 2 * f + hp
                    for jc in range(2):
                        nc.tensor.matmul(
                            out=ops[hp * 64:hp * 64 + 64,
                                    f * 256:(f + 1) * 256],
                            lhsT=vh[r * 2 + jc][:, H * 64:(H + 1) * 64],
                            rhs=EhN[:, H * 512 + jc * 256:
                                    H * 512 + (jc + 1) * 256],
                            start=(jc == 0), stop=(jc == 1))
            copy_ps(ohT[:, :, r * 256:(r + 1) * 256],
                    ops.rearrange("p (f i) -> p f i", f=4))

        # height output projection
        for mc in range(2):
            for nt in range(T // 512):
                ps = psC.tile([128, 512], F32, tag="projC", name="projC")
                for f in range(4):
                    nc.tensor.matmul(
                        out=ps,
                        lhsT=ho_sb[f][:, mc * 128:(mc + 1) * 128],
                        rhs=ohT[:, f, nt * 512:(nt + 1) * 512],
                        start=(f == 0), stop=(f == 3))
                st = stgC.tile([128, 512], F32, tag="stgH", name="stgH")
                copy_ps(st, ps)
                nc.sync.dma_start(
                    out=h_out_t[mc * 128:(mc + 1) * 128,
                                nt * 512:(nt + 1) * 512],
                    in_=st)

    ctx.close()


_NC = None


def _get_nc():
    global _NC
    if _NC is None:
        _NC = build_bass()
    return _NC


def make_in_maps(x, wq_w, wkv_w, wout_w, hq_w, hkv_w, hout_w):
    x4 = np.asarray(x, np.float32).reshape(H_ROWS, W_COLS, D)
    xb = x4.astype(NPBF16)
    wghts = {
        "wq": np.ascontiguousarray(np.asarray(wq_w, np.float32).astype(NPBF16)),
        "wk": np.ascontiguousarray(np.asarray(wkv_w, np.float32)[:, :INNER].astype(NPBF16)),
        "wv": np.ascontiguousarray(np.asarray(wkv_w, np.float32)[:, INNER:].astype(NPBF16)),
        "wo": np.ascontiguousarray(np.asarray(wout_w, np.float32).astype(NPBF16)),
        "hq": np.ascontiguousarray(np.asarray(hq_w, np.float32).astype(NPBF16)),
        "hk": np.ascontiguousarray(np.asarray(hkv_w, np.float32)[:, :INNER].astype(NPBF16)),
        "hv": np.ascontiguousarray(np.asarray(hkv_w, np.float32)[:, INNER:].astype(NPBF16)),
        "ho": np.ascontiguousarray(np.asarray(hout_w, np.float32).astype(NPBF16)),
    }
    in_maps = []
    for c in range(N_CORES):
        xw_c = np.ascontiguousarray(
            xb[:, c * WPC:(c + 1) * WPC, :].transpose(1, 0, 2)
            .reshape(T, D).T)
        xr_c = np.ascontiguousarray(xb[c * RPC:(c + 1) * RPC].reshape(T, D).T)
        m = {"xw": xw_c, "xr": xr_c}
        m.update(wghts)
        in_maps.append(m)
    return in_maps


def assemble_output(results, wout_b, hout_b):
    w_full = np.empty((H_ROWS, W_COLS, D), np.float32)
    h_full = np.empty((H_ROWS, W_COLS, D), np.float32)
    for c in range(N_CORES):
        wt = results[c]["w_out_t"]  # (256, 4096)
        w_full[:, c * WPC:(c + 1) * WPC, :] = \
            wt.T.reshape(WPC, H_ROWS, D).transpose(1, 0, 2)
        ht = results[c]["h_out_t"]
        h_full[c * RPC:(c + 1) * RPC] = ht.T.reshape(RPC, W_COLS, D)
    out = w_full + h_full
    out += (np.asarray(wout_b, np.float32) + np.asarray(hout_b, np.float32))
    return out.reshape(1, H_ROWS * W_COLS, D)


def kernel(x, wq_w, wkv_w, wout_w, wout_b, hq_w, hkv_w, hout_w, hout_b,
           msa_h=H_ROWS, msa_w=W_COLS, **_unused):
    in_maps = make_in_maps(x, wq_w, wkv_w, wout_w, hq_w, hkv_w, hout_w)
    nc = _get_nc()
    res = run_bass_kernel_spmd(nc, in_maps, core_ids=list(range(N_CORES)))
    return assemble_output(res.results, wout_b, hout_b)


# revision 13
# speedup vs baseline: 270.1984x; 1.0066x over previous
"""AlphaFold2 axial (row/column) MSA attention on 8 Trainium2 NeuronCores.

Problem: x (1, 32768, 256) = 128 MSA rows x 256 columns x dim 256.
  - width attention: softmax attention across the 128 rows, independent per
    column (256 independent length-128 sequences), 8 heads x 64.
  - height attention: "tied" attention across the 256 columns: logits are
    summed over all 128 rows, one (256x256) softmax per head shared by all
    rows.

Sharding (8 cores):
  - width: each core owns 32 columns (fully local).
  - height: each core owns 16 rows; per-core partial logits (8,256,256) are
    AllReduce'd (bf16, 1MB) across cores, softmax replicated, attn*V local.

Layout strategy (everything bf16 into the PE, fp32 accumulation):
  - activations feature-major ("xT": features on partitions, tokens on free),
    prepared host-side, so projections and q.k^T need no on-device transpose.
  - scores are computed transposed, S^T = (j, i), by swapping matmul
    operands; softmax denominators are computed with an all-ones stationary
    matmul (partition-dim sum + broadcast in one PE op), normalization via
    reciprocal + multiply; no max-subtraction (logits are ~N(0, 0.1)).
  - attn*V consumes S^T directly and yields o^T feature-major, which feeds
    the output projection; outputs are written feature-major and transposed
    back on the host.

PSUM rules honored here: a matmul accumulation chain must fully finish
before another chain's start=True touches the same PSUM bank (start clears
has_written bank-wide; data values persist).
"""

import sys

for _p in ("/opt/trn_rl_repo",):
    if _p not in sys.path:
        sys.path.append(_p)

import numpy as np
import ml_dtypes

import concourse.bass as bass
import concourse.mybir as mybir
import concourse.tile as tile
from concourse import bacc
from concourse.bass_utils import run_bass_kernel_spmd

BF16 = mybir.dt.bfloat16
F32 = mybir.dt.float32
NPBF16 = ml_dtypes.bfloat16
EXP = mybir.ActivationFunctionType.Exp

N_CORES = 8
H_ROWS = 128          # MSA rows
W_COLS = 256          # sequence length (columns)
D = 256               # model dim
NH = 8                # heads
DH = 64               # head dim
INNER = NH * DH       # 512
WPC = W_COLS // N_CORES   # 32 columns per core
RPC = H_ROWS // N_CORES   # 16 rows per core
T = 4096              # tokens per shard (WPC*H_ROWS == RPC*W_COLS)
SCALE = DH ** -0.5                   # 0.125
TIE_SCALE = SCALE * (H_ROWS ** -0.5)


def _ap(h):
    return h.ap()


def build_bass(loop=1):
    nc = bacc.Bacc("TRN2", target_bir_lowering=False, debug=False,
                   num_devices=N_CORES)

    # ---- per-core I/O ----
    xw = _ap(nc.dram_tensor("xw", [D, T], BF16, kind="ExternalInput"))
    xr = _ap(nc.dram_tensor("xr", [D, T], BF16, kind="ExternalInput"))
    wq = _ap(nc.dram_tensor("wq", [D, INNER], BF16, kind="ExternalInput"))
    wk = _ap(nc.dram_tensor("wk", [D, INNER], BF16, kind="ExternalInput"))
    wv = _ap(nc.dram_tensor("wv", [D, INNER], BF16, kind="ExternalInput"))
    wo = _ap(nc.dram_tensor("wo", [INNER, D], BF16, kind="ExternalInput"))
    hq = _ap(nc.dram_tensor("hq", [D, INNER], BF16, kind="ExternalInput"))
    hk = _ap(nc.dram_tensor("hk", [D, INNER], BF16, kind="ExternalInput"))
    hv = _ap(nc.dram_tensor("hv", [D, INNER], BF16, kind="ExternalInput"))
    ho = _ap(nc.dram_tensor("ho", [INNER, D], BF16, kind="ExternalInput"))
    w_out_t = _ap(nc.dram_tensor("w_out_t", [D, T], F32, kind="ExternalOutput"))
    h_out_t = _ap(nc.dram_tensor("h_out_t", [D, T], F32, kind="ExternalOutput"))

    with tile.TileContext(nc) as tc:
        for it in range(loop):
            # collective buffers must be distinct per unrolled iteration
            cc_in = _ap(nc.dram_tensor(f"cc_in{it}", [128, NH * 512], BF16,
                                       kind="Internal"))
            cc_out = _ap(nc.dram_tensor(f"cc_out{it}", [128, NH * 512], BF16,
                                        kind="Internal", addr_space="Shared"))
            build_tile_kernel(tc, xw, xr, wq, wk, wv, wo, hq, hk, hv, ho,
                              w_out_t, h_out_t, cc_in, cc_out)

    nc.compile()
    return nc


def build_tile_kernel(tc, xw, xr, wq, wk, wv, wo, hq, hk, hv, ho,
                      w_out_t, h_out_t, cc_in, cc_out):
    from contextlib import ExitStack

    nc = tc.nc
    ctx = ExitStack()

    # round-robin PSUM->SBUF copy engine (only ACT/DVE can read PSUM)
    _eng = [0]

    def copy_ps(out, in_):
        _eng[0] ^= 1
        (nc.scalar.copy if _eng[0] else nc.vector.tensor_copy)(out=out, in_=in_)

    consts = ctx.enter_context(tc.tile_pool(name="consts", bufs=1))

    # ---- constants / weights into SBUF ----
    def load_w2(ap_in, name):  # (256, 512) -> 2 chunks (128, 512)
        ts = []
        for kc in range(2):
            t = consts.tile([128, INNER], BF16, name=f"{name}{kc}")
            nc.sync.dma_start(out=t, in_=ap_in[kc * 128:(kc + 1) * 128, :])
            ts.append(t)
        return ts

    def load_w4(ap_in, name):  # (512, 256) -> 4 chunks (128, 256)
        ts = []
        for f in range(4):
            t = consts.tile([128, D], BF16, name=f"{name}{f}")
            nc.sync.dma_start(out=t, in_=ap_in[f * 128:(f + 1) * 128, :])
            ts.append(t)
        return ts

    # phase-A dependencies first so the PE can start ASAP
    hq_sb = load_w2(hq, "hq")
    hk_sb = load_w2(hk, "hk")
    # chunked loads so the first projection matmuls can start early
    xr_sb = []
    for kc in range(2):
        t = consts.tile([128, T], BF16, name=f"xr{kc}")
        xr_sb.append(t)
    for half in range(2):
        for kc in range(2):
            nc.sync.dma_start(
                out=xr_sb[kc][:, half * 2048:(half + 1) * 2048],
                in_=xr[kc * 128:(kc + 1) * 128,
                       half * 2048:(half + 1) * 2048])
    xw_sb = []
    for kc in range(2):
        t = consts.tile([128, T], BF16, name=f"xw{kc}")
        xw_sb.append(t)
    for half in range(2):
        for kc in range(2):
            nc.sync.dma_start(
                out=xw_sb[kc][:, half * 2048:(half + 1) * 2048],
                in_=xw[kc * 128:(kc + 1) * 128,
                       half * 2048:(half + 1) * 2048])
    wq_sb = load_w2(wq, "wq")
    wk_sb = load_w2(wk, "wk")
    wv_sb = load_w2(wv, "wv")
    hv_sb = load_w2(hv, "hv")
    wo_sb = load_w4(wo, "wo")
    ho_sb = load_w4(ho, "ho")

    ones_sb = consts.tile([128, 128], BF16, name="ones")
    nc.vector.memset(ones_sb, 1.0)

    # ---------------------------------------------------------------
    # Phase A: height q/k projections + partial tied logits; AllReduce.
    # dots^T[H](j, i) = sum_r sum_d k[r,j,H,d] q[r,i,H,d]  (j,i = columns)
    # ---------------------------------------------------------------
    dotsAp = ctx.enter_context(tc.tile_pool(name="dotsAp", bufs=1))
    with tc.tile_pool(name="phaseA", bufs=1) as phaseA, \
         tc.tile_pool(name="psA", bufs=3, space="PSUM") as psA, \
         tc.tile_pool(name="psDA", bufs=2, space="PSUM") as psDA:

        def project_fmajor(w_sb, x_sb, pool, name):
            """(feat, tok) = w^T @ x^T -> 4 chunks (128, T) bf16."""
            outs = []
            for f in range(4):
                t = pool.tile([128, T], BF16, name=f"{name}{f}")
                outs.append(t)
                for nt in range(T // 512):
                    ps = psA.tile([128, 512], F32, tag="projA", name="projA")
                    for kc in range(2):
                        nc.tensor.matmul(
                            out=ps,
                            lhsT=w_sb[kc][:, f * 128:(f + 1) * 128],
                            rhs=x_sb[kc][:, nt * 512:(nt + 1) * 512],
                            start=(kc == 0), stop=(kc == 1))
                    copy_ps(t[:, nt * 512:(nt + 1) * 512], ps)
            return outs

        qhT = project_fmajor(hq_sb, xr_sb, phaseA, "qhT")
        khT = project_fmajor(hk_sb, xr_sb, phaseA, "khT")

        # partial dots^T, bf16: (128, [H][jc][i]) free = H*512 + jc*256 + i
        dots_sb = dotsAp.tile([128, NH * 512], BF16, name="dots_sb")
        for f in range(4):
            # heads 2f (free 0:512, bank 0) and 2f+1 (free 512:1024, bank 1)
            dps = psDA.tile([128, 1024], F32, tag="hdots", name="hdots")
            # complete each jc accumulation chain before the next starts in
            # the same bank (start=True clears has_written bank-wide)
            for jc in range(2):
                for r in range(RPC):
                    for hp in range(2):
                        b = hp * 64
                        nc.tensor.matmul(
                            out=dps[:, hp * 512 + jc * 256:
                                    hp * 512 + (jc + 1) * 256],
                            lhsT=khT[f][b:b + 64, r * 256 + jc * 128:
                                        r * 256 + jc * 128 + 128],
                            rhs=qhT[f][b:b + 64, r * 256:(r + 1) * 256],
                            start=(r == 0), stop=(r == RPC - 1))
            copy_ps(dots_sb[:, 2 * f * 512:(2 * f + 2) * 512], dps)

        nc.sync.dma_start(out=cc_in[:, :], in_=dots_sb[:, :])
        nc.gpsimd.collective_compute(
            "AllReduce", mybir.AluOpType.add,
            replica_groups=[list(range(N_CORES))],
            ins=[cc_in.opt()], outs=[cc_out.opt()])

    # ---------------------------------------------------------------
    # Phase B: width attention over this core's 32 columns.
    # Ew slot layout: slot(H) = (H%2)*512 + (H//2)*128
    # ---------------------------------------------------------------
    NCG = 8                      # columns per group
    NGRP = WPC // NCG            # 4 groups
    GT = NCG * 128               # tokens per group (1024)

    with tc.tile_pool(name="phaseB", bufs=1) as phaseB, \
         tc.tile_pool(name="grpB", bufs=2) as grpB, \
         tc.tile_pool(name="colB", bufs=6) as colB, \
         tc.tile_pool(name="stgB", bufs=3) as stgB, \
         tc.tile_pool(name="psW", bufs=3, space="PSUM") as psW, \
         tc.tile_pool(name="psSO", bufs=2, space="PSUM") as psSO:
        psB = psSO

        # o^T accumulator: (128, f, tok) -- chunk f holds heads 2f, 2f+1
        owT = phaseB.tile([128, 4, T], BF16, name="owT")

        def emit_group_proj_chunks(g):
            """Return a list of thunks, each emitting one projection chunk
            (2 matmuls + copy) for group g."""
            tok0 = g * GT
            qwT, kwT, vw = [], [], []
            thunks = []
            for f in range(4):
                for which, lst in ((0, qwT), (1, kwT)):
                    w_sb = (wq_sb, wk_sb)[which]
                    t = grpB.tile([128, GT], BF16, tag=f"qk{which}{f}",
                                  name=f"qk{which}{f}")
                    lst.append(t)
                    for nt in range(GT // 512):
                        def th(w_sb=w_sb, t=t, nt=nt, f=f, tok0=tok0):
                            ps = psB.tile([128, 512], F32, tag="so512",
                                          name="projB")
                            for kc in range(2):
                                nc.tensor.matmul(
                                    out=ps,
                                    lhsT=w_sb[kc][:, f * 128:(f + 1) * 128],
                                    rhs=xw_sb[kc][:, tok0 + nt * 512:
                                                  tok0 + (nt + 1) * 512],
                                    start=(kc == 0), stop=(kc == 1))
                            copy_ps(t[:, nt * 512:(nt + 1) * 512], ps)
                        thunks.append(th)
            for ci in range(NCG):
                t = grpB.tile([128, INNER], BF16, tag=f"vw{ci}",
                              name=f"vw{ci}")
                vw.append(t)
                def th(t=t, ci=ci, tok0=tok0):
                    ps = psB.tile([128, 512], F32, tag="so512", name="projB")
                    for kc in range(2):
                        nc.tensor.matmul(
                            out=ps,
                            lhsT=xw_sb[kc][:, tok0 + ci * 128:
                                           tok0 + (ci + 1) * 128],
                            rhs=wv_sb[kc],
                            start=(kc == 0), stop=(kc == 1))
                    copy_ps(t, ps)
                thunks.append(th)
            return (qwT, kwT, vw), thunks

        def emit_col(qwT, kwT, vw, g, ci):
            tok0 = g * GT
            c0 = ci * 128  # token offset within group
            # scores^T halves: dps[hp][:, f*128:...] = S^T[H=2f+hp]
            dps = [psW.tile([128, 512], F32, tag=f"wdots{hp}",
                            name=f"wdots{hp}") for hp in range(2)]
            for f in range(4):
                for hp in range(2):
                    b = hp * 64
                    nc.tensor.matmul(
                        out=dps[hp][:, f * 128:(f + 1) * 128],
                        lhsT=kwT[f][b:b + 64, c0:c0 + 128],
                        rhs=qwT[f][b:b + 64, c0:c0 + 128],
                        start=True, stop=True)
            Ew = colB.tile([128, 1024], BF16, tag="Ew", name="Ew")
            Binv = colB.tile([128, 1024], F32, tag="Binv", name="Binv")
            EwN = colB.tile([128, 1024], BF16, tag="EwN", name="EwN")
            ops = psSO.tile([128, 512], F32, tag="so512", name="opsW")
            for hp in range(2):
                hb = hp * 512
                nc.scalar.activation(
                    out=Ew[:, hb:hb + 512], in_=dps[hp],
                    func=EXP, scale=SCALE)
                # denominator sums broadcast to all partitions, written
                # in-place over the consumed scores bank
                nc.tensor.matmul(out=dps[hp], lhsT=ones_sb,
                                 rhs=Ew[:, hb:hb + 512],
                                 start=True, stop=True)
                nc.vector.reciprocal_approx_fast(
                    out=Binv[:, hb:hb + 512], in_=dps[hp])
                nc.gpsimd.tensor_mul(out=EwN[:, hb:hb + 512],
                                     in0=Ew[:, hb:hb + 512],
                                     in1=Binv[:, hb:hb + 512])
                # attn * V for the 4 heads of this half
                # (o^T chunk f: head 2f @ part 0:64, 2f+1 @ 64:128)
                for f in range(4):
                    H = 2 * f + hp
                    nc.tensor.matmul(
                        out=ops[hp * 64:hp * 64 + 64,
                                f * 128:(f + 1) * 128],
                        lhsT=vw[ci][:, H * 64:(H + 1) * 64],
                        rhs=EwN[:, hb + f * 128:hb + (f + 1) * 128],
                        start=True, stop=True)
            copy_ps(owT[:, :, tok0 + c0:tok0 + c0 + 128],
                    ops.rearrange("p (f i) -> p f i", f=4))

        # software pipeline: group g's columns interleave with group g+1's
        # projections so the PE always has independent work queued
        cur_tiles, thunks = emit_group_proj_chunks(0)
        for th in thunks:
            th()
        for g in range(NGRP):
            nxt = None
            if g + 1 < NGRP:
                nxt_tiles, nxt_thunks = emit_group_proj_chunks(g + 1)
                nxt = iter(nxt_thunks)
                per_col = (len(nxt_thunks) + NCG - 1) // NCG
            for ci in range(NCG):
                emit_col(*cur_tiles, g, ci)
                if nxt is not None:
                    for _ in range(per_col):
                        th = next(nxt, None)
                        if th is not None:
                            th()
            if nxt is not None:
                for th in nxt:
                    th()
                cur_tiles = nxt_tiles

        # width output projection for this group's tokens
            for mc in range(2):
                for gt in range(GT // 512):
                    nt = (tok0 // 512) + gt
                    ps = psB.tile([128, 512], F32, tag="projB", name="oprojW")
                    for f in range(4):
                        nc.tensor.matmul(
                            out=ps,
                            lhsT=wo_sb[f][:, mc * 128:(mc + 1) * 128],
                            rhs=owT[:, f, nt * 512:(nt + 1) * 512],
                            start=(f == 0), stop=(f == 3))
                    st = stgB.tile([128, 512], F32, tag="stgW", name="stgW")
                    copy_ps(st, ps)
                    nc.sync.dma_start(
                        out=w_out_t[mc * 128:(mc + 1) * 128,
                                    nt * 512:(nt + 1) * 512],
                        in_=st)

    # ---------------------------------------------------------------
    # Phase C: height attention finish (after AllReduce).
    # ---------------------------------------------------------------
    with tc.tile_pool(name="phaseC", bufs=1) as phaseC, \
         tc.tile_pool(name="stgC", bufs=3) as stgC, \
         tc.tile_pool(name="psC", bufs=2, space="PSUM") as psC, \
         tc.tile_pool(name="psSC", bufs=2, space="PSUM") as psSC, \
         tc.tile_pool(name="psOC", bufs=2, space="PSUM") as psOC:

        # v (token-major) for the row shard: 32 chunks (128, 512)
        vh = []
        for rc in range(32):
            t = phaseC.tile([128, INNER], BF16, name=f"vh{rc}")
            vh.append(t)
            ps = psC.tile([128, 512], F32, tag="projC", name="projC")
            for kc in range(2):
                nc.tensor.matmul(
                    out=ps,
                    lhsT=xr_sb[kc][:, rc * 128:(rc + 1) * 128],
                    rhs=hv_sb[kc],
                    start=(kc == 0), stop=(kc == 1))
            copy_ps(t, ps)

        dotsr = phaseC.tile([128, NH * 512], BF16, name="dotsr")
        nc.sync.dma_start(out=dotsr[:, :], in_=cc_out[:, :])

        Eh = phaseC.tile([128, NH * 512], BF16, name="Eh")
        for H in range(NH):
            nc.scalar.activation(out=Eh[:, H * 512:(H + 1) * 512],
                                 in_=dotsr[:, H * 512:(H + 1) * 512],
                                 func=EXP, scale=TIE_SCALE)
        # denominators: B_H(i) = sum over both j-chunks and partitions
        BinvH = phaseC.tile([128, NH * 256], F32, name="BinvH")
        for f in range(4):
            bps = psSC.tile([128, 512], F32, tag="bsumH", name="bsumH")
            for hp in range(2):
                H = 2 * f + hp
                for jc in range(2):
                    nc.tensor.matmul(
                        out=bps[:, hp * 256:(hp + 1) * 256],
                        lhsT=ones_sb,
                        rhs=Eh[:, H * 512 + jc * 256:
                               H * 512 + (jc + 1) * 256],
                        start=(jc == 0), stop=(jc == 1))
                nc.vector.reciprocal_approx_fast(
                    out=BinvH[:, H * 256:(H + 1) * 256],
                    in_=bps[:, hp * 256:(hp + 1) * 256])
        EhN = phaseC.tile([128, NH * 512], BF16, name="EhN")
        for H in range(NH):
            for jc in range(2):
                nc.gpsimd.tensor_mul(
                    out=EhN[:, H * 512 + jc * 256: H * 512 + (jc + 1) * 256],
                    in0=Eh[:, H * 512 + jc * 256: H * 512 + (jc + 1) * 256],
                    in1=BinvH[:, H * 256:(H + 1) * 256])

        # attn * V per row -> o^T chunks; ohT free = (f, r*256 + i)
        ohT = phaseC.tile([128, 4, T], BF16, name="ohT")
        for r in range(RPC):
            ops = psOC.tile([128, 1024], F32, tag="opsH", name="opsH")
            for f in range(4):
                for hp in range(2):
                    H = 2 * f + hp
                    for jc in range(2):
                        nc.tensor.matmul(
                            out=ops[hp * 64:hp * 64 + 64,
                                    f * 256:(f + 1) * 256],
                            lhsT=vh[r * 2 + jc][:, H * 64:(H + 1) * 64],
                            rhs=EhN[:, H * 512 + jc * 256:
                                    H * 512 + (jc + 1) * 256],
                            start=(jc == 0), stop=(jc == 1))
            copy_ps(ohT[:, :, r * 256:(r + 1) * 256],
                    ops.rearrange("p (f i) -> p f i", f=4))

            if r % 2 == 1:
                # height output projection for the two finished rows
                nt = r // 2
                for mc in range(2):
                    ps = psC.tile([128, 512], F32, tag="projC", name="oprojH")
                    for f in range(4):
                        nc.tensor.matmul(
                            out=ps,
                            lhsT=ho_sb[f][:, mc * 128:(mc + 1) * 128],
                            rhs=ohT[:, f, nt * 512:(nt + 1) * 512],
                            start=(f == 0), stop=(f == 3))
                    st = stgC.tile([128, 512], F32, tag="stgH", name="stgH")
                    copy_ps(st, ps)
                    nc.sync.dma_start(
                        out=h_out_t[mc * 128:(mc + 1) * 128,
                                    nt * 512:(nt + 1) * 512],
                        in_=st)

    ctx.close()


_NC = None


def _get_nc():
    global _NC
    if _NC is None:
        _NC = build_bass()
    return _NC


def make_in_maps(x, wq_w, wkv_w, wout_w, hq_w, hkv_w, hout_w):
    x4 = np.asarray(x, np.float32).reshape(H_ROWS, W_COLS, D)
    xb = x4.astype(NPBF16)
    wghts = {
        "wq": np.ascontiguousarray(np.asarray(wq_w, np.float32).astype(NPBF16)),
        "wk": np.ascontiguousarray(np.asarray(wkv_w, np.float32)[:, :INNER].astype(NPBF16)),
        "wv": np.ascontiguousarray(np.asarray(wkv_w, np.float32)[:, INNER:].astype(NPBF16)),
        "wo": np.ascontiguousarray(np.asarray(wout_w, np.float32).astype(NPBF16)),
        "hq": np.ascontiguousarray(np.asarray(hq_w, np.float32).astype(NPBF16)),
        "hk": np.ascontiguousarray(np.asarray(hkv_w, np.float32)[:, :INNER].astype(NPBF16)),
        "hv": np.ascontiguousarray(np.asarray(hkv_w, np.float32)[:, INNER:].astype(NPBF16)),
        "ho": np.ascontiguousarray(np.asarray(hout_w, np.float32).astype(NPBF16)),
    }
    in_maps = []
    for c in range(N_CORES):
        xw_c = np.ascontiguousarray(
            xb[:, c * WPC:(c + 1) * WPC, :].transpose(1, 0, 2)
            .reshape(T, D).T)
        xr_c = np.ascontiguousarray(xb[c * RPC:(c + 1) * RPC].reshape(T, D).T)
        m = {"xw": xw_c, "xr": xr_c}
        m.update(wghts)
        in_maps.append(m)
    return in_maps


def assemble_output(results, wout_b, hout_b):
    w_full = np.empty((H_ROWS, W_COLS, D), np.float32)
    h_full = np.empty((H_ROWS, W_COLS, D), np.float32)
    for c in range(N_CORES):
        wt = results[c]["w_out_t"]  # (256, 4096)
        w_full[:, c * WPC:(c + 1) * WPC, :] = \
            wt.T.reshape(WPC, H_ROWS, D).transpose(1, 0, 2)
        ht = results[c]["h_out_t"]
        h_full[c * RPC:(c + 1) * RPC] = ht.T.reshape(RPC, W_COLS, D)
    out = w_full + h_full
    out += (np.asarray(wout_b, np.float32) + np.asarray(hout_b, np.float32))
    return out.reshape(1, H_ROWS * W_COLS, D)


def kernel(x, wq_w, wkv_w, wout_w, wout_b, hq_w, hkv_w, hout_w, hout_b,
           msa_h=H_ROWS, msa_w=W_COLS, **_unused):
    in_maps = make_in_maps(x, wq_w, wkv_w, wout_w, hq_w, hkv_w, hout_w)
    nc = _get_nc()
    res = run_bass_kernel_spmd(nc, in_maps, core_ids=list(range(N_CORES)))
    return assemble_output(res.results, wout_b, hout_b)
